# revision 32
# baseline (speedup 1.0000x reference)
"""DoRA multihead attention TRN2 kernel, v2: software-pipelined phases.

Per core (4 heads, one batch):
  lead:   DMA loads, k/q projections for head-pair 0 (e-tile m=0)
  h0:     S(h0,j)+exp rounds; v-proj(j) and m=1 projections as PE filler
  h1..h3: S(h,j)+exp rounds; AV(h-1,j) accumulating in 4 PSUM chunks
  tail:   AV(h3), out-projection, DMA out

PSUM budget: S double-buffer 2x[128,1024] (4 banks) + 4 work slots [128,512].
"""
import sys
if "/opt/trn_rl_repo" not in sys.path:
    sys.path.insert(0, "/opt/trn_rl_repo")

import numpy as np
import ml_dtypes
from contextlib import ExitStack

import concourse.bass as bass
import concourse.tile as tile
from concourse import bacc, mybir

bf = ml_dtypes.bfloat16
BF16, F32 = mybir.dt.bfloat16, mybir.dt.float32
ActFn = mybir.ActivationFunctionType

T = 2048
E = 1024
HD = 64
NHL = 4
HSL = NHL * HD          # 256
VW = NHL * (HD + 1)     # 260
NT = T // 128           # 16
NF = E // 128           # 8
NI = T // 512           # 4


def build_body(tc, qT, wq, wk, wv, wo, bq, bk, y):
    nc = tc.nc
    with ExitStack() as ctx:
        const = ctx.enter_context(tc.tile_pool(name="const", bufs=1))
        rpool = ctx.enter_context(tc.tile_pool(name="recips", bufs=3))
        bpool = ctx.enter_context(tc.tile_pool(name="bcasts", bufs=3))
        ypool = ctx.enter_context(tc.tile_pool(name="yout", bufs=2))
        pss = ctx.enter_context(tc.tile_pool(name="pss", bufs=2, space="PSUM"))
        psw = ctx.enter_context(tc.tile_pool(name="psw", bufs=2, space="PSUM"))
        # pT pool A (heads 0, 2): opened before the input pool so LIFO
        # close order holds when the input pool is released after h0.
        ppool = ctx.enter_context(tc.tile_pool(name="pTA", bufs=32))
        wpool_cm = tc.tile_pool(name="wts", bufs=1)
        wpool = wpool_cm.__enter__()

        # ---- input loads (scoped pool, freed before pT pool B opens) ----
        # ordered so the first q-projection's dependencies land first:
        # bq, wq, qk wave0, then wk/bk (k-proj), remaining waves, wv, wo
        # one coalesced DMA per weight matrix (per-DMA dispatch is ~0.6us):
        # [E, C] -> [128, NF*C] with tile f at columns [f*C, (f+1)*C)
        # wq and qT-wave-0 land in interleaved f-halves so the first
        # projection group's f=0..3 matmuls start one half-DMA earlier
        wq_all = wpool.tile([128, NF * HSL], BF16, tag="wq", name="wq_all")
        qk_all = wpool.tile([128, NF * T], BF16, tag="qTs", name="qk_all")
        qk = [qk_all[:, f * T:(f + 1) * T] for f in range(NF)]

        def qk_wave(n):
            nc.sync.dma_start(
                qk_all[:].rearrange("p (f c) -> p f c", f=NF)[:, :, n * 512:(n + 1) * 512],
                qT[:, n * 512:(n + 1) * 512].rearrange("(f p) c -> p f c", p=128))

        nc.sync.dma_start(
            wq_all[:], wq.rearrange("(f p) c -> p f c", p=128))
        bq_s = const.tile([128, 2], F32, tag="bq")
        nc.sync.dma_start(bq_s[:], bq[:])
        for fh in range(2):
            nc.sync.dma_start(
                qk_all[:].rearrange("p (f c) -> p f c", f=NF)[:, fh * 4:(fh + 1) * 4, 0:512],
                qT[fh * 512:(fh + 1) * 512, 0:512].rearrange("(f p) c -> p f c", p=128))
        wk_all = wpool.tile([128, NF * HSL], BF16, tag="wk", name="wk_all")
        nc.sync.dma_start(
            wk_all[:], wk.rearrange("(f p) c -> p f c", p=128))
        bk_s = const.tile([128, 2], F32, tag="bk")
        nc.sync.dma_start(bk_s[:], bk[:])
        qk_wave(1)
        qk_wave(2)
        qk_wave(3)
        wv_all = wpool.tile([128, NF * VW], BF16, tag="wv", name="wv_all")
        nc.sync.dma_start(
            wv_all[:], wv.rearrange("(f p) c -> p f c", p=128))
        wq_s = [wq_all[:, f * HSL:(f + 1) * HSL] for f in range(NF)]
        wk_s = [wk_all[:, f * HSL:(f + 1) * HSL] for f in range(NF)]
        wv_s = [wv_all[:, f * VW:(f + 1) * VW] for f in range(NF)]
        wo_all = const.tile([128, 2 * E], BF16, tag="wo", name="wo_all")
        nc.sync.dma_start(
            wo_all[:], wo.rearrange("(g p) c -> p g c", p=128))
        wo_s = [wo_all[:, g * E:(g + 1) * E] for g in range(2)]

        # per-head projection tiles padded to K=128 (rows 64-127 stay zero)
        # so S matmuls run in the same 128-row PE tiling mode as everything
        # else -- mode switches drain the TensorE pipeline on HW.
        kTp = [const.tile([128, T], BF16, tag=f"kTph{h}", name=f"kTph{h}") for h in range(NHL)]
        qTp = [const.tile([128, T], BF16, tag=f"qTph{h}", name=f"qTph{h}") for h in range(NHL)]
        for h in range(NHL):
            nc.gpsimd.memset(kTp[h][64:128, :], 0.0)
            nc.gpsimd.memset(qTp[h][64:128, :], 0.0)
        v_sb = [const.tile([128, VW], BF16, tag=f"v{i}", name=f"v{i}") for i in range(NT)]
        oT = [const.tile([128, T], BF16, tag=f"oT{g}", name=f"oT{g}") for g in range(2)]

        def proj_qk_group(w_tiles, bias_s, out_tiles, m, n):
            ps = psw.tile([128, 512], F32, tag="ps", name="ps")
            for f in range(NF):
                nc.tensor.matmul(
                    ps[:], w_tiles[f][:, m * 128:(m + 1) * 128],
                    qk[f][:, n * 512:(n + 1) * 512],
                    start=(f == 0), stop=(f == NF - 1))
            for hh in range(2):
                nc.vector.tensor_scalar_add(
                    out_tiles[2 * m + hh][0:64, n * 512:(n + 1) * 512],
                    ps[hh * 64:(hh + 1) * 64, :],
                    bias_s[hh * 64:(hh + 1) * 64, m:m + 1])

        def proj_v(it):
            ps = psw.tile([128, 512], F32, tag="ps", name="ps")
            pv = ps[:, 0:VW]
            for f in range(NF):
                nc.tensor.matmul(
                    pv, qk[f][:, it * 128:(it + 1) * 128], wv_s[f][:],
                    start=(f == 0), stop=(f == NF - 1))
            nc.vector.tensor_copy(v_sb[it][:], pv)
            # denominator-ones column per head (wv has zeros there; bias_v is
            # folded into the host-side output bias)
            nc.vector.memset(
                v_sb[it].rearrange("p (h c) -> p h c", h=NHL)[:, :, HD:HD + 1], 1.0)

        def s_exp(h, j, pT_tiles):
            # S.T half-tiles [j_block 128, i 1024] + exp -> bf16
            for half in range(2):
                ps = pss.tile([128, 1024], F32, tag="s", name="s")
                for n2 in range(2):
                    n = half * 2 + n2
                    nc.tensor.matmul(
                        ps[:, n2 * 512:(n2 + 1) * 512],
                        kTp[h][:, j * 128:(j + 1) * 128],
                        qTp[h][:, n * 512:(n + 1) * 512],
                        start=True, stop=True)
                pool = ppool if h % 2 == 0 else ppoolB
                pt = pool.tile([128, 1024], BF16, tag="pT", name=f"pT_{h}_{j}_{half}")
                nc.scalar.activation(pt[:], ps[:], ActFn.Exp, scale=0.125)
                pT_tiles[j][half] = pt

        def av_normalize(h, n, av_n):
            g, po = h // 2, (h % 2) * 64
            recip = rpool.tile([1, 512], F32, tag="recip", name="recip")
            nc.vector.reciprocal(recip[:], av_n[64:65, :])
            rbc = bpool.tile([64, 512], F32, tag="rbc", name="rbc")
            nc.gpsimd.partition_broadcast(rbc[:], recip[:])
            nc.vector.tensor_mul(
                oT[g][po:po + 64, n * 512:(n + 1) * 512],
                av_n[0:64, :], rbc[:])

        # ---- lead: m=0 q and k projections interleaved per qT wave, so PE
        # work tracks DMA wave arrival (a q-group alone consumes a wave in
        # ~1.7us while the next wave needs ~2.9us to land).
        for n in range(NI):
            proj_qk_group(wq_s, bq_s, qTp, 0, n)
            proj_qk_group(wk_s, bk_s, kTp, 0, n)

        pT = {h: [[None, None] for _ in range(NT)] for h in range(NHL)}

        # ---- h0 phase: S(h0) rounds + v-proj + m=1 projections ----
        extra_groups = [(wk_s, bk_s, kTp, 1, n) for n in range(NI)] + \
                       [(wq_s, bq_s, qTp, 1, n) for n in range(NI)]
        for j in range(NT):
            s_exp(0, j, pT[0])
            proj_v(j)
            if j < len(extra_groups):
                proj_qk_group(*extra_groups[j])

        wpool_cm.__exit__(None, None, None)
        ppoolB = ctx.enter_context(tc.tile_pool(name="pTB", bufs=32))

        # ---- h1..h3 phases: S(h) + AV(h-1) ----
        # AV matmuls run chunk-major (all 16 j-tiles of output chunk n, then
        # chunk n+1) so each chunk normalizes as soon as it completes: at most
        # 2 AV accumulators are ever live (vs 4 chunk-parallel), freeing 2
        # PSUM banks for the 3-deep S/exp rotation, and the normalize work
        # spreads across the phase instead of bursting at its end.
        # The first AV matmuls are deferred by 2 S-rounds (PSUM slot reuse);
        # rounds 2-3 run a double budget to catch up.
        av_cur = [None]

        def av_step(hprev, c):
            n, jj = c // NT, c % NT
            if jj == 0:
                av_cur[0] = psw.tile([128, 512], F32, tag="ps",
                                     name=f"av{hprev}_{n}")
            nc.tensor.matmul(
                av_cur[0][0:HD + 1, :],
                v_sb[jj][:, hprev * 65:hprev * 65 + 65],
                pT[hprev][jj][n // 2][:, (n % 2) * 512:(n % 2 + 1) * 512],
                start=(jj == 0), stop=(jj == NT - 1))
            if jj == NT - 1:
                av_normalize(hprev, n, av_cur[0])

        # During h3, head 3's own AV for query-chunks 0-1 also runs j-major
        # in two persistent PSUM slots (tag avq): the S+AV(h2) rounds alone
        # (1706ns) trail the 2-exp ACT rate (1992ns), and this both fills
        # that deficit and removes half of av_h3 from the tail.
        h3 = NHL - 1
        avq = [psw.tile([128, 512], F32, tag="avq", name=f"avq{n}", bufs=2)
               for n in range(2)]

        def avq_step(jj):
            for n in range(2):
                nc.tensor.matmul(
                    avq[n][0:HD + 1, :],
                    v_sb[jj][:, h3 * 65:h3 * 65 + 65],
                    pT[h3][jj][n // 2][:, (n % 2) * 512:(n % 2 + 1) * 512],
                    start=(jj == 0), stop=(jj == NT - 1))

        for h in range(1, NHL):
            av_c = 0
            for j in range(NT):
                s_exp(h, j, pT[h])
                if j >= 2:
                    budget = 8 if j < 4 else 4
                    for _ in range(budget):
                        if av_c < NI * NT:
                            av_step(h - 1, av_c)
                            av_c += 1
                    if h == h3 and j >= 2:
                        avq_step(j - 2)
            if h == h3:
                for jj in range(NT - 2, NT):
                    avq_step(jj)

        # ---- tail: chunks 0-1 of head 3 are already accumulated (avq) and
        # just normalize; AV of chunks 2-3 interleaves with out-projection so
        # the PE never waits on a chunk's normalize chain (DVE recip -> Pool
        # bcast -> DVE mul, ~1us).
        def av_h3(n):
            for jj in range(NT):
                av_step(h3, n * NT + jj)

        def op_chunk(n):
            # out-projection for the 4 token tiles covered by chunk n: both
            # e-chunks accumulate in one 2-bank PSUM tile, evacuated in halves
            # split across the (tail-idle) Scalar and Vector engines
            for it in range(4 * n, 4 * n + 4):
                # whole-tile evac on one engine, alternating DVE/ACT per it:
                # two writers into one yt tile would WW-serialize, and per-half
                # DMAs double the HWDGE descriptor load (it scales with rows,
                # not bytes) which starves the buffer rotation
                # ACT-heavy split (11/5): DVE also runs the normalize chains
                # in the tail, so an even split leaves it borderline-saturated
                use_dve = it % 2 == 0
                yt = ypool.tile([128, E], mybir.dt.float16,
                                tag=f"y{int(use_dve)}", name="yt")
                ps = pss.tile([128, 1024], F32, tag="s", name="yps")
                for ec in range(2):
                    for g in range(2):
                        nc.tensor.matmul(
                            ps[:, ec * 512:(ec + 1) * 512],
                            oT[g][:, it * 128:(it + 1) * 128],
                            wo_s[g][:, ec * 512:(ec + 1) * 512],
                            start=(g == 0), stop=(g == 1))
                if use_dve:
                    nc.vector.tensor_copy(yt[:], ps[:])
                else:
                    nc.scalar.copy(yt[:], ps[:])
                nc.sync.dma_start(y[it * 128:(it + 1) * 128, :], yt[:])

        av_normalize(h3, 0, avq[0])
        av_normalize(h3, 1, avq[1])
        av_h3(2)
        op_chunk(0)
        av_h3(3)
        op_chunk(1)
        op_chunk(2)
        op_chunk(3)


def build_nc(num_devices=8, reps=1):
    nc = bacc.Bacc("TRN2", target_bir_lowering=False, debug=False,
                   num_devices=num_devices)
    qT = nc.dram_tensor("qT", [E, T], BF16, kind="ExternalInput").ap()
    wq = nc.dram_tensor("wq", [E, HSL], BF16, kind="ExternalInput").ap()
    wk = nc.dram_tensor("wk", [E, HSL], BF16, kind="ExternalInput").ap()
    wv = nc.dram_tensor("wv", [E, VW], BF16, kind="ExternalInput").ap()
    wo = nc.dram_tensor("wo", [HSL, E], BF16, kind="ExternalInput").ap()
    bq = nc.dram_tensor("bq", [128, 2], F32, kind="ExternalInput").ap()
    bk = nc.dram_tensor("bk", [128, 2], F32, kind="ExternalInput").ap()
    y = nc.dram_tensor("y", [T, E], mybir.dt.float16, kind="ExternalOutput").ap()
    with tile.TileContext(nc) as tc:
        for _ in range(reps):
            build_body(tc, qT, wq, wk, wv, wo, bq, bk, y)
    nc.compile()
    return nc


# host-side prep/gather identical to v1


# ---------------- host-side shard prep / gather ----------------

def eff_weight(mag, dirw, Am, Bm):
    Vu = dirw.astype(np.float32) + Bm.astype(np.float32) @ Am.astype(np.float32)
    c = np.float32(mag) / (np.linalg.norm(Vu) + np.float32(1e-8))
    return (c * Vu).astype(np.float32)


def make_in_maps(inputs):
    query = np.asarray(inputs["query"], np.float32)
    Wq = eff_weight(inputs["mag_q"], inputs["dir_q"], inputs["A_q"], inputs["B_q"])
    Wv = eff_weight(inputs["mag_v"], inputs["dir_v"], inputs["A_v"], inputs["B_v"])
    k_w = np.asarray(inputs["k_w"], np.float32)
    out_w = np.asarray(inputs["out_w"], np.float32)
    bias_q = np.asarray(inputs["bias_q"], np.float32)
    k_b = np.asarray(inputs["k_b"], np.float32)
    bias_v = np.asarray(inputs["bias_v"], np.float32)

    qT_b = [np.ascontiguousarray(query[:, b, :].T).astype(bf) for b in range(2)]
    WqT, WkT, WvT, WoT = Wq.T, k_w.T, Wv.T, out_w.T

    in_maps = []
    for c in range(8):
        b, h0 = c // 4, (c % 4) * 4
        cols = slice(h0 * HD, h0 * HD + HSL)
        wv_aug = np.zeros((E, VW), np.float32)
        for hl in range(NHL):
            src = slice((h0 + hl) * HD, (h0 + hl + 1) * HD)
            wv_aug[:, hl * 65:hl * 65 + HD] = WvT[:, src]
        in_maps.append({
            "qT": qT_b[b],
            "wq": np.ascontiguousarray(WqT[:, cols]).astype(bf),
            "wk": np.ascontiguousarray(WkT[:, cols]).astype(bf),
            "wv": wv_aug.astype(bf),
            "wo": np.ascontiguousarray(WoT[cols, :]).astype(bf),
            "bq": bias_q[cols].reshape(2, 128).T.copy(),
            "bk": k_b[cols].reshape(2, 128).T.copy(),
        })
    return in_maps


def gather_output(results, inputs):
    # per-core partials may be fp16 (halves the output-DMA tail); sum in fp32.
    # bias_v never enters the device v-path: softmax weights sum to 1, so its
    # contribution to the output is exactly (bias_v @ out_w.T), folded here.
    out_b = np.asarray(inputs["out_b"], np.float32) + \
        np.asarray(inputs["bias_v"], np.float32) @ np.asarray(
            inputs["out_w"], np.float32).T
    out = np.empty((T, 2, E), np.float32)
    for b in range(2):
        acc = results[4 * b]["y"].astype(np.float32)
        for c in range(4 * b + 1, 4 * b + 4):
            acc += results[c]["y"].astype(np.float32)
        out[:, b, :] = acc + out_b
    return out


# ---------------- public entry point ----------------
# The compiled module and the jitted PJRT executable are cached at module
# level so repeat kernel() calls skip build/trace/lower (~seconds saved).

_CACHE = {}


class _Exec:
    def __init__(self, nc, n_cores=8):
        import jax
        from jax.sharding import Mesh, PartitionSpec
        from jax.experimental.shard_map import shard_map
        from concourse import mybir as _mb
        from concourse.bass2jax import (
            _bass_exec_p, install_neuronx_cc_hook, partition_id_tensor)

        install_neuronx_cc_hook()
        self.jax = jax
        self.n_cores = n_cores
        pname = nc.partition_id_tensor.name if nc.partition_id_tensor else None
        in_names, out_names, out_avals = [], [], []
        for alloc in nc.m.functions[0].allocations:
            if not isinstance(alloc, _mb.MemoryLocationSet):
                continue
            name = alloc.memorylocations[0].name
            if alloc.kind == "ExternalInput":
                if name != pname:
                    in_names.append(name)
            elif alloc.kind == "ExternalOutput":
                out_avals.append(jax.core.ShapedArray(
                    tuple(alloc.tensor_shape), _mb.dt.np(alloc.dtype)))
                out_names.append(name)
        self.in_names, self.out_names, self.out_avals = in_names, out_names, out_avals
        all_names = in_names + out_names + ([pname] if pname else [])

        def _body(*args):
            operands = list(args)
            if pname is not None:
                operands.append(partition_id_tensor())
            return tuple(_bass_exec_p.bind(
                *operands, out_avals=tuple(out_avals), in_names=tuple(all_names),
                out_names=tuple(out_names), lowering_input_output_aliases=(),
                sim_require_finite=True, sim_require_nnan=True, nc=nc))

        devices = jax.devices()[:n_cores]
        import numpy as _np
        self.mesh = Mesh(_np.asarray(devices), ("core",))
        nin = len(in_names) + len(out_names)
        self.fn = jax.jit(
            shard_map(_body, mesh=self.mesh, in_specs=(PartitionSpec("core"),) * nin,
                      out_specs=(PartitionSpec("core"),) * len(out_names),
                      check_rep=False),
            keep_unused=True)
        self.sharding = jax.sharding.NamedSharding(self.mesh, PartitionSpec("core"))

    def run(self, in_maps):
        jax = self.jax
        n = self.n_cores
        concat_in = [
            np.concatenate([np.asarray(in_maps[c][name]) for c in range(n)], axis=0)
            for name in self.in_names
        ]
        zeros = [np.zeros((n * a.shape[0], *a.shape[1:]), a.dtype)
                 for a in self.out_avals]
        args = [jax.device_put(x, self.sharding) for x in concat_in + zeros]
        outs = self.fn(*args)
        jax.block_until_ready(outs)
        return [
            {name: np.asarray(outs[i]).reshape(n, *self.out_avals[i].shape)[c]
             for i, name in enumerate(self.out_names)}
            for c in range(n)
        ]


def _get_exec():
    if "exec" not in _CACHE:
        _CACHE["exec"] = _Exec(build_nc(num_devices=8, reps=1))
    return _CACHE["exec"]


def kernel(**inputs):
    """Full-input, full-output DoRA multihead attention on 8 NeuronCores.

    Shards 32 (batch, head) units across 8 cores (4 heads each); host
    reconstructs the (tiny) DoRA effective weights, pre-transposes the
    per-batch query to bf16, and sums the 4 per-core output partials per
    batch (+ out_b) at the end.
    """
    import time as _time

    inputs = {k: np.asarray(v) for k, v in inputs.items()}
    in_maps = make_in_maps(inputs)
    last_err = None
    for _attempt in range(6):
        try:
            ex = _get_exec()
            results = ex.run(in_maps)
            break
        except Exception as e:  # transient device errors observed on axon
            last_err = e
            _CACHE.pop("exec", None)
            _time.sleep(4.0 * (_attempt + 1))
    else:
        raise last_err
    return gather_output(results, inputs)

